# revision 42
# baseline (speedup 1.0000x reference)
"""Trainium2 kernel for nn_AdaptiveTransformation (ragged V/UV time resample).

Reference semantics: per batch item, run-length V/UV segmentation of
vu_mask drives a two-pass bilinear time resample T_in=4096 -> T_out=6144
(frequency resample is identity since F_out == F_in == 128). All
data-dependence composes into per-output-column gather indices + weights
(4 taps per column, monotone, locally clustered).

Kernel strategy (pure batch data-parallelism, 8 items per NeuronCore):
  host: compute index maps (eager jax on CPU — bit-exact vs reference),
        convert to banded form: per output tile of TILE_T columns, a
        window of W consecutive mel time-frames plus a [W, TILE_T] bf16
        selection/weight matrix S with <=4 nnz per column.
  device: out_tile[f, t] = sum_r win[r, f] * S[r, t] — K=W matmuls on the
        TensorEngine (gather+blend fused), PSUM -> SBUF copy on DVE,
        sequential DMA in/out (bf16 both ways).

Primary mode "fused64": TILE_T = W = 64; two tiles packed per 128
partitions (even tile rows on partitions 0-63 run on PE row-tile T0,
odd rows on 64-127 run on T8; the two 64-row tiles execute concurrently
into separate PSUM banks). win and S ship as one merged "ws" tensor,
two items per DMA, all transfers on the sync HWDGE ring. Falls back to
"plain128_{1,2,3}" (TILE_T = 128, W = 128 * n_chunks, K=128 matmuls)
if the data's window span exceeds 64.

Measured on 8 axon-attached TRN2 cores: ~95 us per invocation
(in-NEFF For_i repeat differencing), rel err ~2.2e-3 vs the reference
(bf16 data + weights, f32 PSUM accumulation).
"""
import numpy as np

VOICED_RATIO = 0.7
B = 64
F = 128
T_IN = 4096
T_OUT = 6144
N_CORES = 8
ITEMS_PER_CORE = B // N_CORES  # 8

_jax = None
_cpu = None


def _lazy_jax():
    global _jax, _cpu
    if _jax is None:
        import jax

        _jax = jax
        _cpu = jax.devices("cpu")[0]
    return _jax, _cpu


# ---------------------------------------------------------------------------
# Host-side index computation (mirrors the reference _time_maps verbatim;
# run eagerly on jax-CPU so float32 tie-breaking matches the reference).
# ---------------------------------------------------------------------------


def _time_maps(jnp, jox, mask, T_out):
    T_in = mask.shape[0]
    ratio = VOICED_RATIO / (1.0 - VOICED_RATIO)
    change = jnp.concatenate(
        [jnp.ones((1,), jnp.int32), (mask[1:] != mask[:-1]).astype(jnp.int32)]
    )
    seg_id = jnp.cumsum(change) - 1
    n_segs = seg_id[-1] + 1
    seg_len = jox.segment_sum(jnp.ones((T_in,), jnp.float32), seg_id, num_segments=T_in)
    seg_voiced = jox.segment_sum(mask.astype(jnp.float32), seg_id, num_segments=T_in) > 0
    len_v = jnp.sum(mask).astype(jnp.float32)
    len_uv = T_in - len_v
    both = (len_v > 0) & (len_uv > 0)
    rv = jnp.where(
        both,
        T_out / (len_v + len_uv / ratio),
        jnp.where(
            len_v > 0, T_out / jnp.maximum(len_v, 1.0), T_out / jnp.maximum(len_uv, 1.0)
        ),
    )
    ruv = jnp.where(both, rv / ratio, rv)
    seg_scale = jnp.where(seg_voiced, rv, ruv)
    idx = jnp.arange(T_in)
    valid = idx < n_segs
    last = idx == (n_segs - 1)
    tgt = jnp.maximum(1.0, jnp.round(seg_scale * seg_len)).astype(jnp.int32)
    tgt = jnp.where(valid & ~last, tgt, 0)
    tgt = jnp.where(last, jnp.maximum(1, T_out - jnp.sum(tgt)), tgt)
    cum_end = jnp.cumsum(tgt)
    cum_start = cum_end - tgt
    T_total = cum_end[-1]
    seg_start = jnp.cumsum(seg_len) - seg_len

    def concat_to_input(k):
        s = jnp.minimum(jnp.searchsorted(cum_end, k, side="right"), T_in - 1)
        L = seg_len[s]
        TL = jnp.maximum(tgt[s].astype(jnp.float32), 1.0)
        kl = (k - cum_start[s]).astype(jnp.float32)
        src = jnp.maximum((kl + 0.5) * (L / TL) - 0.5, 0.0)
        x0 = jnp.floor(src)
        w = src - x0
        x1 = jnp.minimum(x0 + 1.0, L - 1.0)
        base = seg_start[s]
        return (base + x0).astype(jnp.int32), (base + x1).astype(jnp.int32), w

    t = jnp.arange(T_out, dtype=jnp.float32)
    sc = jnp.maximum((t + 0.5) * (T_total.astype(jnp.float32) / T_out) - 0.5, 0.0)
    k0 = jnp.floor(sc).astype(jnp.int32)
    k1 = jnp.minimum(k0 + 1, T_total - 1)
    lam = sc - k0.astype(jnp.float32)
    a0, a1, wa = concat_to_input(k0)
    b0, b1, wb = concat_to_input(k1)
    return a0, a1, wa, b0, b1, wb, lam


def compute_index_maps(vu_mask: np.ndarray, T_out: int):
    jax, cpu = _lazy_jax()
    import jax.numpy as jnp

    with jax.default_device(cpu):
        a0, a1, wa, b0, b1, wb, lam = jax.vmap(
            lambda m: _time_maps(jnp, jax.ops, m, T_out)
        )(jnp.asarray(vu_mask))
        return tuple(np.asarray(x) for x in (a0, a1, wa, b0, b1, wb, lam))


# ---------------------------------------------------------------------------
# Banded-form construction
# ---------------------------------------------------------------------------


def _tap_arrays(vu_mask):
    a0, a1, wa, b0, b1, wb, lam = compute_index_maps(vu_mask, T_OUT)
    w0 = (1.0 - lam) * (1.0 - wa)
    w1 = (1.0 - lam) * wa
    w2 = lam * (1.0 - wb)
    w3 = lam * wb
    idxs = np.stack([a0, a1, b0, b1], 0)  # (4, B, T_OUT)
    wts = np.stack([w0, w1, w2, w3], 0).astype(np.float32)
    return idxs, wts


def _banded_s(idxs, wts, tile_t, W):
    """Per-tile window starts + dense banded S.

    Returns starts (B, G) int32 and S (B, G, W, tile_t) float32, where
    G = T_OUT // tile_t. Returns None if some tile's span exceeds W.
    """
    G = T_OUT // tile_t
    it = idxs.reshape(4, B, G, tile_t)
    wt = wts.reshape(4, B, G, tile_t)
    min_tap = it.min(axis=(0, 3))
    max_tap = it.max(axis=(0, 3))
    if int((max_tap - min_tap).max()) + 1 > W:
        return None, None
    starts = np.minimum(min_tap, T_IN - W).astype(np.int32)
    loc = it - starts[None, :, :, None]
    assert loc.min() >= 0 and loc.max() < W
    bg = np.arange(B)[:, None, None] * G + np.arange(G)[None, :, None]
    lin = ((bg * W)[None] + loc) * tile_t + np.arange(tile_t)[None, None, None, :]
    S = np.bincount(
        lin.ravel(),
        weights=wt.ravel().astype(np.float64),
        minlength=B * G * W * tile_t,
    ).astype(np.float32)
    return starts, S.reshape(B, G, W, tile_t)


def _gather_windows(mel_T, starts, W):
    """win (B, G, W, F) f32 from mel_T (B, T_IN, F)."""
    row_idx = starts[:, :, None] + np.arange(W)[None, None, :]
    return mel_T[np.arange(B)[:, None, None], row_idx]


def build_device_inputs(mel: np.ndarray, vu_mask: np.ndarray):
    """Returns (mode, win, S): win/S flattened per-item device layouts."""
    import ml_dtypes

    idxs, wts = _tap_arrays(vu_mask)
    mel_T = np.ascontiguousarray(mel[:, 0].transpose(0, 2, 1))  # (B, T_IN, F)

    # (32-wide tiling is blocked: ISA operand base partitions are limited
    # to {0, 32, 64}, so the 4th 32-row PE tile cannot be addressed.)
    starts, S = _banded_s(idxs, wts, 64, 64)
    if starts is not None:
        # fused64: pair64 tiling with win+S merged into one tensor.
        win = _gather_windows(mel_T, starts, 64)  # (B, 96, 64, F)
        win = win.reshape(B, 48, 2, 64, F).transpose(0, 2, 3, 1, 4)
        win = np.ascontiguousarray(win).astype(ml_dtypes.bfloat16)  # (B,2,64,48,F)
        S = S.reshape(B, 48, 2, 64, 64).transpose(0, 2, 3, 1, 4)
        S = np.ascontiguousarray(S).astype(ml_dtypes.bfloat16)  # (B,2,64,48,64)
        ws = np.concatenate(
            [win.reshape(B, 128, 48 * F), S.reshape(B, 128, 48 * 64)], axis=2
        )
        return ("fused64", np.ascontiguousarray(ws), None)

    for n_chunks in (1, 2, 3):
        W = 128 * n_chunks
        starts, S = _banded_s(idxs, wts, 128, W)
        if starts is not None:
            G = 48
            win = _gather_windows(mel_T, starts, W)  # (B, 48, W, F)
            win = win.reshape(B, G, n_chunks, 128, F).transpose(0, 3, 1, 2, 4)
            win = np.ascontiguousarray(win).astype(ml_dtypes.bfloat16)
            S = S.reshape(B, G, n_chunks, 128, 128).transpose(0, 3, 1, 2, 4)
            S = np.ascontiguousarray(S).astype(ml_dtypes.bfloat16)
            return (
                f"plain128_{n_chunks}",
                win.reshape(B, 128, G * n_chunks * F),
                S.reshape(B, 128, G * n_chunks * 128),
            )
    raise AssertionError("window span exceeds 384 rows — unexpected input")


# ---------------------------------------------------------------------------
# Device program
# ---------------------------------------------------------------------------

_PROGRAM_CACHE: dict = {}


def _split_multiwait_instructions(nc):
    """The stock walrus codegen accepts only one sync wait per instruction.
    Tile emits several on join points; move the extras onto single-wait
    NoOps inserted immediately before, on the same engine."""
    import concourse.mybir as mybir

    for f in nc.m.functions:
        for bb in f.blocks:
            insts = bb.instructions  # live list
            i = 0
            while i < len(insts):
                ins = insts[i]
                si = ins.sync_info
                waits = list(si.on_wait) if si is not None and si.on_wait else []
                if len(waits) > 1:
                    for k, w in enumerate(waits[:-1]):
                        nop = mybir.InstNoOp(name=f"{ins.name}-ws{k}")
                        nop.engine = ins.engine
                        nop.sync_info = mybir.SyncInfo(on_wait=[w], on_update=[])
                        insts.insert(i, nop)
                        i += 1
                    si.on_wait = waits[-1:]
                i += 1


def _mode_sizes(mode):
    if mode == "quad32":
        return 48 * F, 48 * 32  # per-partition win / s elements
    if mode in ("pair64", "fused64"):
        return 48 * F, 48 * 64
    n_chunks = int(mode.split("_")[1])
    return 48 * n_chunks * F, 48 * n_chunks * 128


def build_program(mode: str, n_reps: int = 1, internal_io: bool = False):
    """Build the per-core Bass program. n_reps > 1 wraps the body in a
    For_i repeat loop. internal_io=True replaces the big external tensors
    with internal DRAM tensors (garbage data) plus tiny dummy params, so
    timing harnesses avoid the per-call host<->device transfer cost."""
    import concourse.bass as bass
    import concourse.mybir as mybir
    import concourse.tile as tile

    win_elems, s_elems = _mode_sizes(mode)
    nc = bass.Bass()
    fused = mode == "fused64"
    if internal_io:
        nc.declare_dram_parameter("tin", [1, 1], mybir.dt.float32, isOutput=False)
        if fused:
            ws_ext = nc.dram_tensor(
                "ws", [ITEMS_PER_CORE, 128, win_elems + s_elems], mybir.dt.bfloat16
            )
        else:
            win_ext = nc.dram_tensor(
                "win", [ITEMS_PER_CORE, 128, win_elems], mybir.dt.bfloat16
            )
            s_ext = nc.dram_tensor(
                "s", [ITEMS_PER_CORE, 128, s_elems], mybir.dt.bfloat16
            )
        out_ext = nc.dram_tensor("out", [ITEMS_PER_CORE, 128, T_OUT], mybir.dt.bfloat16)
        nc.declare_dram_parameter("tout", [1, 1], mybir.dt.float32, isOutput=True)
    else:
        if fused:
            ws_ext = nc.declare_dram_parameter(
                "ws",
                [ITEMS_PER_CORE, 128, win_elems + s_elems],
                mybir.dt.bfloat16,
                isOutput=False,
            )
        else:
            win_ext = nc.declare_dram_parameter(
                "win",
                [ITEMS_PER_CORE, 128, win_elems],
                mybir.dt.bfloat16,
                isOutput=False,
            )
            s_ext = nc.declare_dram_parameter(
                "s", [ITEMS_PER_CORE, 128, s_elems], mybir.dt.bfloat16, isOutput=False
            )
        out_ext = nc.declare_dram_parameter(
            "out", [ITEMS_PER_CORE, 128, T_OUT], mybir.dt.bfloat16, isOutput=True
        )

    with tile.TileContext(nc) as tc:
        with (
            tc.tile_pool(name="io", bufs=2 if mode == "fused64" else 3) as io_pool,
            tc.tile_pool(name="outp", bufs=2) as out_pool,
            tc.tile_pool(
                name="psum", bufs=2 if mode == "quad32" else 4, space="PSUM"
            ) as psum_pool,
        ):

            def item_quad32(item):
                # K=32 row-tiled matmuls: tiles T0/T4/T8/T12 own SBUF row
                # quarters and run concurrently, each into its own PSUM bank.
                win_t = io_pool.tile([128, 48, F], mybir.dt.bfloat16, tag="win")
                s_t = io_pool.tile([128, 48, 32], mybir.dt.bfloat16, tag="s")
                win_src = win_ext[item].rearrange("p (g c) -> p g c", g=48)
                s_src = s_ext[item].rearrange("p (g t) -> p g t", g=48)
                nc.sync.dma_start(win_t[:], win_src[:])
                nc.sync.dma_start(s_t[:], s_src[:])
                out_t = out_pool.tile([128, 48, 4, 32], mybir.dt.bfloat16, tag="out")
                GRP = 16  # quad-groups per psum generation (16 * 32 = 512 cols)
                for gg in range(0, 48, GRP):
                    pss = [
                        psum_pool.tile(
                            [128, GRP * 32],
                            mybir.dt.float32,
                            tag=f"p{q}",
                            name=f"ps{q}",
                        )
                        for q in range(4)
                    ]
                    for q in range(4):
                        for i in range(GRP):
                            gq = gg + i
                            nc.tensor.matmul(
                                pss[q][:, i * 32 : (i + 1) * 32],
                                lhsT=win_t[32 * q : 32 * (q + 1), gq, :],
                                rhs=s_t[32 * q : 32 * (q + 1), gq, :],
                                start=True,
                                stop=True,
                            )
                    for q in range(4):
                        nc.vector.tensor_copy(
                            out_t[:, gg : gg + GRP, q, :],
                            pss[q][:].rearrange("p (i t) -> p i t", i=GRP),
                        )
                nc.sync.dma_start(
                    out_ext[item], out_t[:].rearrange("p g h t -> p (g h t)")
                )

            def duo_fused64(item0):
                # pair64 tiling; win+S arrive as one tensor, two items per DMA.
                WE = 48 * F
                ws_t = io_pool.tile(
                    [128, 2, WE + 48 * 64], mybir.dt.bfloat16, tag="ws"
                )
                nc.sync.dma_start(
                    ws_t[:], ws_ext[item0 : item0 + 2].rearrange("i p e -> p i e")
                )
                out_t = out_pool.tile(
                    [128, 2, 48, 2, 64], mybir.dt.bfloat16, tag="out"
                )
                GRP = 8
                for it in range(2):
                    win_v = ws_t[:, it, :WE].rearrange("p (g c) -> p g c", g=48)
                    s_v = ws_t[:, it, WE:].rearrange("p (g t) -> p g t", g=48)
                    for gg in range(0, 48, GRP):
                        ps_a = psum_pool.tile(
                            [128, GRP * 64], mybir.dt.float32, tag="pa"
                        )
                        ps_b = psum_pool.tile(
                            [128, GRP * 64], mybir.dt.float32, tag="pb"
                        )
                        for q in range(GRP):
                            gp = gg + q
                            nc.tensor.matmul(
                                ps_a[:, q * 64 : (q + 1) * 64],
                                lhsT=win_v[0:64, gp, :],
                                rhs=s_v[0:64, gp, :],
                                start=True,
                                stop=True,
                            )
                            nc.tensor.matmul(
                                ps_b[:, q * 64 : (q + 1) * 64],
                                lhsT=win_v[64:128, gp, :],
                                rhs=s_v[64:128, gp, :],
                                start=True,
                                stop=True,
                            )
                        nc.vector.tensor_copy(
                            out_t[:, it, gg : gg + GRP, 0, :],
                            ps_a[:].rearrange("p (q t) -> p q t", q=GRP),
                        )
                        nc.vector.tensor_copy(
                            out_t[:, it, gg : gg + GRP, 1, :],
                            ps_b[:].rearrange("p (q t) -> p q t", q=GRP),
                        )
                nc.sync.dma_start(
                    out_ext[item0 : item0 + 2].rearrange("i p t -> p i t"),
                    out_t[:].rearrange("p i g h t -> p i (g h t)"),
                )

            def item_pair64(item):
                # K=64 row-tiled matmuls: tile T0 (SBUF rows 0-63, even
                # output 64-col halves) and T8 (rows 64-127, odd halves)
                # run concurrently but MUST target different PSUM banks.
                win_t = io_pool.tile([128, 48, F], mybir.dt.bfloat16, tag="win")
                s_t = io_pool.tile([128, 48, 64], mybir.dt.bfloat16, tag="s")
                win_src = win_ext[item].rearrange("p (g c) -> p g c", g=48)
                s_src = s_ext[item].rearrange("p (g t) -> p g t", g=48)
                nc.sync.dma_start(win_t[:], win_src[:])
                nc.sync.dma_start(s_t[:], s_src[:])
                out_t = out_pool.tile([128, 48, 2, 64], mybir.dt.bfloat16, tag="out")
                GRP = 8  # pairs per psum bank-pair (8 * 64 cols = 512 each)
                for gg in range(0, 48, GRP):
                    ps_a = psum_pool.tile([128, GRP * 64], mybir.dt.float32, tag="pa")
                    ps_b = psum_pool.tile([128, GRP * 64], mybir.dt.float32, tag="pb")
                    for q in range(GRP):
                        gp = gg + q
                        nc.tensor.matmul(
                            ps_a[:, q * 64 : (q + 1) * 64],
                            lhsT=win_t[0:64, gp, :],
                            rhs=s_t[0:64, gp, :],
                            start=True,
                            stop=True,
                        )
                        nc.tensor.matmul(
                            ps_b[:, q * 64 : (q + 1) * 64],
                            lhsT=win_t[64:128, gp, :],
                            rhs=s_t[64:128, gp, :],
                            start=True,
                            stop=True,
                        )
                    nc.vector.tensor_copy(
                        out_t[:, gg : gg + GRP, 0, :],
                        ps_a[:].rearrange("p (q t) -> p q t", q=GRP),
                    )
                    nc.vector.tensor_copy(
                        out_t[:, gg : gg + GRP, 1, :],
                        ps_b[:].rearrange("p (q t) -> p q t", q=GRP),
                    )
                nc.sync.dma_start(
                    out_ext[item], out_t[:].rearrange("p g h t -> p (g h t)")
                )

            def pair_plain128(item0, n_chunks):
                """Process items item0, item0+1 with batched DMAs."""
                NCk = n_chunks
                win_t = io_pool.tile(
                    [128, 2, 48, NCk, F], mybir.dt.bfloat16, tag="win"
                )
                s_t = io_pool.tile(
                    [128, 2, 48, NCk, 128], mybir.dt.bfloat16, tag="s"
                )
                win_src = win_ext[item0 : item0 + 2].rearrange(
                    "i p (g j c) -> p i g j c", g=48, j=NCk
                )
                s_src = s_ext[item0 : item0 + 2].rearrange(
                    "i p (g j t) -> p i g j t", g=48, j=NCk
                )
                nc.sync.dma_start(win_t[:], win_src)
                nc.sync.dma_start(s_t[:], s_src)
                out_t = out_pool.tile([128, 2, T_OUT], mybir.dt.bfloat16, tag="out")
                GRP = 4
                for it in range(2):
                    for gg in range(0, 48, GRP):
                        ps = psum_pool.tile(
                            [128, GRP * 128], mybir.dt.float32, tag="ps"
                        )
                        for q in range(GRP):
                            g = gg + q
                            for j in range(NCk):
                                nc.tensor.matmul(
                                    ps[:, q * 128 : (q + 1) * 128],
                                    lhsT=win_t[:, it, g, j, :],
                                    rhs=s_t[:, it, g, j, :],
                                    start=(j == 0),
                                    stop=(j == NCk - 1),
                                )
                        nc.vector.tensor_copy(
                            out_t[:, it, gg * 128 : (gg + GRP) * 128], ps[:]
                        )
                nc.sync.dma_start(
                    out_ext[item0 : item0 + 2].rearrange("i p t -> p i t"), out_t[:]
                )

            def body(_iv=None):
                if mode == "fused64":
                    for item0 in range(0, ITEMS_PER_CORE, 2):
                        duo_fused64(item0)
                elif mode == "pair64":
                    for item in range(ITEMS_PER_CORE):
                        item_pair64(item)
                else:
                    for item0 in range(0, ITEMS_PER_CORE, 2):
                        pair_plain128(item0, int(mode.split("_")[1]))

            if n_reps == 1:
                body()
            else:
                with tc.For_i(0, n_reps, 1) as iv:
                    body(iv)

    _split_multiwait_instructions(nc)
    return nc


def get_program(mode: str, n_reps: int = 1, internal_io: bool = False):
    key = (mode, n_reps, internal_io)
    if key not in _PROGRAM_CACHE:
        _PROGRAM_CACHE[key] = build_program(mode, n_reps, internal_io)
    return _PROGRAM_CACHE[key]


# ---------------------------------------------------------------------------
# Entry point
# ---------------------------------------------------------------------------


def _prep(mel, vu_mask):
    mode, win_flat, s_flat = build_device_inputs(np.asarray(mel), np.asarray(vu_mask))
    in_maps = []
    for c in range(N_CORES):
        sl = slice(c * ITEMS_PER_CORE, (c + 1) * ITEMS_PER_CORE)
        if mode == "fused64":
            in_maps.append({"ws": np.ascontiguousarray(win_flat[sl])})
        else:
            in_maps.append(
                {
                    "win": np.ascontiguousarray(win_flat[sl]),
                    "s": np.ascontiguousarray(s_flat[sl]),
                }
            )
    return in_maps, mode


def run_on_device(in_maps, mode, n_reps: int = 1, trace: bool = False):
    from concourse.bass_utils import run_bass_kernel_spmd

    nc = get_program(mode, n_reps)
    return run_bass_kernel_spmd(nc, in_maps, core_ids=list(range(N_CORES)), trace=trace)


def run_timing(mode, n_reps: int = 1):
    """Run the internal-IO variant (garbage data, tiny transfers)."""
    from concourse.bass_utils import run_bass_kernel_spmd

    nc = get_program(mode, n_reps, internal_io=True)
    dummy = np.zeros((1, 1), np.float32)
    in_maps = [{"tin": dummy} for _ in range(N_CORES)]
    return run_bass_kernel_spmd(nc, in_maps, core_ids=list(range(N_CORES)))


def kernel(mel, vu_mask, F_out, T_out):
    mel = np.asarray(mel)
    vu_mask = np.asarray(vu_mask)
    assert int(F_out) == F and int(T_out) == T_OUT
    assert mel.shape == (B, 1, F, T_IN) and vu_mask.shape == (B, T_IN)

    in_maps, mode = _prep(mel, vu_mask)
    res = run_on_device(in_maps, mode)
    out = np.concatenate(
        [np.asarray(res.results[c]["out"]) for c in range(N_CORES)], axis=0
    )
    return out.reshape(B, 1, F, T_OUT).astype(np.float32)


# revision 46
# speedup vs baseline: 1.1334x; 1.1334x over previous
"""Trainium2 kernel for nn_AdaptiveTransformation (ragged V/UV time resample).

Reference semantics: per batch item, run-length V/UV segmentation of
vu_mask drives a two-pass bilinear time resample T_in=4096 -> T_out=6144
(frequency resample is identity since F_out == F_in == 128). All
data-dependence composes into per-output-column gather indices + weights
(4 taps per column, monotone, locally clustered).

Kernel strategy (pure batch data-parallelism, 8 items per NeuronCore):
  host: compute index maps (eager jax on CPU — bit-exact vs reference),
        convert to banded form: per output tile of TILE_T columns, a
        window of W consecutive mel time-frames plus a [W, TILE_T] bf16
        selection/weight matrix S with <=4 nnz per column.
  device: out_tile[f, t] = sum_r win[r, f] * S[r, t] — K=W matmuls on the
        TensorEngine (gather+blend fused), PSUM -> SBUF copy on DVE,
        sequential DMA in/out (bf16 both ways).

Primary mode "fused64": TILE_T = W = 64; two tiles packed per 128
partitions (even tile rows on partitions 0-63 run on PE row-tile T0,
odd rows on 64-127 run on T8; the two 64-row tiles execute concurrently
into separate PSUM banks). win and S ship as one merged "ws" tensor,
two items per DMA, all transfers on the sync HWDGE ring. Falls back to
"plain128_{1,2,3}" (TILE_T = 128, W = 128 * n_chunks, K=128 matmuls)
if the data's window span exceeds 64.

PSUM evacuation: 16 pair-tiles accumulate into one two-bank PSUM tile
per row-tile (pa/pb), double-buffered (2 tags x 2 bufs x 2 banks = all 8
banks), evacuated as single [128, 1024] DVE copies — smaller PSUM groups
with deeper rotation serialize PE against DVE and cost ~25%.

Measured on 8 axon-attached TRN2 cores: ~70 us per invocation
(in-NEFF For_i repeat differencing), rel err ~2.2e-3 vs the reference
(bf16 data + weights, f32 PSUM accumulation).
"""
import numpy as np

VOICED_RATIO = 0.7
B = 64
F = 128
T_IN = 4096
T_OUT = 6144
N_CORES = 8
ITEMS_PER_CORE = B // N_CORES  # 8

_jax = None
_cpu = None


def _lazy_jax():
    global _jax, _cpu
    if _jax is None:
        import jax

        _jax = jax
        _cpu = jax.devices("cpu")[0]
    return _jax, _cpu


# ---------------------------------------------------------------------------
# Host-side index computation (mirrors the reference _time_maps verbatim;
# run eagerly on jax-CPU so float32 tie-breaking matches the reference).
# ---------------------------------------------------------------------------


def _time_maps(jnp, jox, mask, T_out):
    T_in = mask.shape[0]
    ratio = VOICED_RATIO / (1.0 - VOICED_RATIO)
    change = jnp.concatenate(
        [jnp.ones((1,), jnp.int32), (mask[1:] != mask[:-1]).astype(jnp.int32)]
    )
    seg_id = jnp.cumsum(change) - 1
    n_segs = seg_id[-1] + 1
    seg_len = jox.segment_sum(jnp.ones((T_in,), jnp.float32), seg_id, num_segments=T_in)
    seg_voiced = jox.segment_sum(mask.astype(jnp.float32), seg_id, num_segments=T_in) > 0
    len_v = jnp.sum(mask).astype(jnp.float32)
    len_uv = T_in - len_v
    both = (len_v > 0) & (len_uv > 0)
    rv = jnp.where(
        both,
        T_out / (len_v + len_uv / ratio),
        jnp.where(
            len_v > 0, T_out / jnp.maximum(len_v, 1.0), T_out / jnp.maximum(len_uv, 1.0)
        ),
    )
    ruv = jnp.where(both, rv / ratio, rv)
    seg_scale = jnp.where(seg_voiced, rv, ruv)
    idx = jnp.arange(T_in)
    valid = idx < n_segs
    last = idx == (n_segs - 1)
    tgt = jnp.maximum(1.0, jnp.round(seg_scale * seg_len)).astype(jnp.int32)
    tgt = jnp.where(valid & ~last, tgt, 0)
    tgt = jnp.where(last, jnp.maximum(1, T_out - jnp.sum(tgt)), tgt)
    cum_end = jnp.cumsum(tgt)
    cum_start = cum_end - tgt
    T_total = cum_end[-1]
    seg_start = jnp.cumsum(seg_len) - seg_len

    def concat_to_input(k):
        s = jnp.minimum(jnp.searchsorted(cum_end, k, side="right"), T_in - 1)
        L = seg_len[s]
        TL = jnp.maximum(tgt[s].astype(jnp.float32), 1.0)
        kl = (k - cum_start[s]).astype(jnp.float32)
        src = jnp.maximum((kl + 0.5) * (L / TL) - 0.5, 0.0)
        x0 = jnp.floor(src)
        w = src - x0
        x1 = jnp.minimum(x0 + 1.0, L - 1.0)
        base = seg_start[s]
        return (base + x0).astype(jnp.int32), (base + x1).astype(jnp.int32), w

    t = jnp.arange(T_out, dtype=jnp.float32)
    sc = jnp.maximum((t + 0.5) * (T_total.astype(jnp.float32) / T_out) - 0.5, 0.0)
    k0 = jnp.floor(sc).astype(jnp.int32)
    k1 = jnp.minimum(k0 + 1, T_total - 1)
    lam = sc - k0.astype(jnp.float32)
    a0, a1, wa = concat_to_input(k0)
    b0, b1, wb = concat_to_input(k1)
    return a0, a1, wa, b0, b1, wb, lam


def compute_index_maps(vu_mask: np.ndarray, T_out: int):
    jax, cpu = _lazy_jax()
    import jax.numpy as jnp

    with jax.default_device(cpu):
        a0, a1, wa, b0, b1, wb, lam = jax.vmap(
            lambda m: _time_maps(jnp, jax.ops, m, T_out)
        )(jnp.asarray(vu_mask))
        return tuple(np.asarray(x) for x in (a0, a1, wa, b0, b1, wb, lam))


# ---------------------------------------------------------------------------
# Banded-form construction
# ---------------------------------------------------------------------------


def _tap_arrays(vu_mask):
    a0, a1, wa, b0, b1, wb, lam = compute_index_maps(vu_mask, T_OUT)
    w0 = (1.0 - lam) * (1.0 - wa)
    w1 = (1.0 - lam) * wa
    w2 = lam * (1.0 - wb)
    w3 = lam * wb
    idxs = np.stack([a0, a1, b0, b1], 0)  # (4, B, T_OUT)
    wts = np.stack([w0, w1, w2, w3], 0).astype(np.float32)
    return idxs, wts


def _banded_s(idxs, wts, tile_t, W):
    """Per-tile window starts + dense banded S.

    Returns starts (B, G) int32 and S (B, G, W, tile_t) float32, where
    G = T_OUT // tile_t. Returns None if some tile's span exceeds W.
    """
    G = T_OUT // tile_t
    it = idxs.reshape(4, B, G, tile_t)
    wt = wts.reshape(4, B, G, tile_t)
    min_tap = it.min(axis=(0, 3))
    max_tap = it.max(axis=(0, 3))
    if int((max_tap - min_tap).max()) + 1 > W:
        return None, None
    starts = np.minimum(min_tap, T_IN - W).astype(np.int32)
    loc = it - starts[None, :, :, None]
    assert loc.min() >= 0 and loc.max() < W
    bg = np.arange(B)[:, None, None] * G + np.arange(G)[None, :, None]
    lin = ((bg * W)[None] + loc) * tile_t + np.arange(tile_t)[None, None, None, :]
    S = np.bincount(
        lin.ravel(),
        weights=wt.ravel().astype(np.float64),
        minlength=B * G * W * tile_t,
    ).astype(np.float32)
    return starts, S.reshape(B, G, W, tile_t)


def _gather_windows(mel_T, starts, W):
    """win (B, G, W, F) f32 from mel_T (B, T_IN, F)."""
    row_idx = starts[:, :, None] + np.arange(W)[None, None, :]
    return mel_T[np.arange(B)[:, None, None], row_idx]


def build_device_inputs(mel: np.ndarray, vu_mask: np.ndarray):
    """Returns (mode, win, S): win/S flattened per-item device layouts."""
    import ml_dtypes

    idxs, wts = _tap_arrays(vu_mask)
    mel_T = np.ascontiguousarray(mel[:, 0].transpose(0, 2, 1))  # (B, T_IN, F)

    # (32-wide tiling is blocked: ISA operand base partitions are limited
    # to {0, 32, 64}, so the 4th 32-row PE tile cannot be addressed.)
    starts, S = _banded_s(idxs, wts, 64, 64)
    if starts is not None:
        # fused64: pair64 tiling with win+S merged into one tensor.
        win = _gather_windows(mel_T, starts, 64)  # (B, 96, 64, F)
        win = win.reshape(B, 48, 2, 64, F).transpose(0, 2, 3, 1, 4)
        win = np.ascontiguousarray(win).astype(ml_dtypes.bfloat16)  # (B,2,64,48,F)
        S = S.reshape(B, 48, 2, 64, 64).transpose(0, 2, 3, 1, 4)
        S = np.ascontiguousarray(S).astype(ml_dtypes.bfloat16)  # (B,2,64,48,64)
        ws = np.concatenate(
            [win.reshape(B, 128, 48 * F), S.reshape(B, 128, 48 * 64)], axis=2
        )
        return ("fused64", np.ascontiguousarray(ws), None)

    for n_chunks in (1, 2, 3):
        W = 128 * n_chunks
        starts, S = _banded_s(idxs, wts, 128, W)
        if starts is not None:
            G = 48
            win = _gather_windows(mel_T, starts, W)  # (B, 48, W, F)
            win = win.reshape(B, G, n_chunks, 128, F).transpose(0, 3, 1, 2, 4)
            win = np.ascontiguousarray(win).astype(ml_dtypes.bfloat16)
            S = S.reshape(B, G, n_chunks, 128, 128).transpose(0, 3, 1, 2, 4)
            S = np.ascontiguousarray(S).astype(ml_dtypes.bfloat16)
            return (
                f"plain128_{n_chunks}",
                win.reshape(B, 128, G * n_chunks * F),
                S.reshape(B, 128, G * n_chunks * 128),
            )
    raise AssertionError("window span exceeds 384 rows — unexpected input")


# ---------------------------------------------------------------------------
# Device program
# ---------------------------------------------------------------------------

_PROGRAM_CACHE: dict = {}


def _split_multiwait_instructions(nc):
    """The stock walrus codegen accepts only one sync wait per instruction.
    Tile emits several on join points; move the extras onto single-wait
    NoOps inserted immediately before, on the same engine."""
    import concourse.mybir as mybir

    for f in nc.m.functions:
        for bb in f.blocks:
            insts = bb.instructions  # live list
            i = 0
            while i < len(insts):
                ins = insts[i]
                si = ins.sync_info
                waits = list(si.on_wait) if si is not None and si.on_wait else []
                if len(waits) > 1:
                    for k, w in enumerate(waits[:-1]):
                        nop = mybir.InstNoOp(name=f"{ins.name}-ws{k}")
                        nop.engine = ins.engine
                        nop.sync_info = mybir.SyncInfo(on_wait=[w], on_update=[])
                        insts.insert(i, nop)
                        i += 1
                    si.on_wait = waits[-1:]
                i += 1


def _mode_sizes(mode):
    if mode == "quad32":
        return 48 * F, 48 * 32  # per-partition win / s elements
    if mode in ("pair64", "fused64"):
        return 48 * F, 48 * 64
    n_chunks = int(mode.split("_")[1])
    return 48 * n_chunks * F, 48 * n_chunks * 128


def build_program(mode: str, n_reps: int = 1, internal_io: bool = False):
    """Build the per-core Bass program. n_reps > 1 wraps the body in a
    For_i repeat loop. internal_io=True replaces the big external tensors
    with internal DRAM tensors (garbage data) plus tiny dummy params, so
    timing harnesses avoid the per-call host<->device transfer cost."""
    import concourse.bass as bass
    import concourse.mybir as mybir
    import concourse.tile as tile

    win_elems, s_elems = _mode_sizes(mode)
    nc = bass.Bass()
    fused = mode == "fused64"
    if internal_io:
        nc.declare_dram_parameter("tin", [1, 1], mybir.dt.float32, isOutput=False)
        if fused:
            ws_ext = nc.dram_tensor(
                "ws", [ITEMS_PER_CORE, 128, win_elems + s_elems], mybir.dt.bfloat16
            )
        else:
            win_ext = nc.dram_tensor(
                "win", [ITEMS_PER_CORE, 128, win_elems], mybir.dt.bfloat16
            )
            s_ext = nc.dram_tensor(
                "s", [ITEMS_PER_CORE, 128, s_elems], mybir.dt.bfloat16
            )
        out_ext = nc.dram_tensor("out", [ITEMS_PER_CORE, 128, T_OUT], mybir.dt.bfloat16)
        nc.declare_dram_parameter("tout", [1, 1], mybir.dt.float32, isOutput=True)
    else:
        if fused:
            ws_ext = nc.declare_dram_parameter(
                "ws",
                [ITEMS_PER_CORE, 128, win_elems + s_elems],
                mybir.dt.bfloat16,
                isOutput=False,
            )
        else:
            win_ext = nc.declare_dram_parameter(
                "win",
                [ITEMS_PER_CORE, 128, win_elems],
                mybir.dt.bfloat16,
                isOutput=False,
            )
            s_ext = nc.declare_dram_parameter(
                "s", [ITEMS_PER_CORE, 128, s_elems], mybir.dt.bfloat16, isOutput=False
            )
        out_ext = nc.declare_dram_parameter(
            "out", [ITEMS_PER_CORE, 128, T_OUT], mybir.dt.bfloat16, isOutput=True
        )

    with tile.TileContext(nc) as tc:
        with (
            tc.tile_pool(name="io", bufs=2 if mode == "fused64" else 3) as io_pool,
            tc.tile_pool(name="outp", bufs=2) as out_pool,
            tc.tile_pool(
                name="psum", bufs=2 if mode == "fused64" else 4, space="PSUM"
            ) as psum_pool,
        ):

            def item_quad32(item):
                # K=32 row-tiled matmuls: tiles T0/T4/T8/T12 own SBUF row
                # quarters and run concurrently, each into its own PSUM bank.
                win_t = io_pool.tile([128, 48, F], mybir.dt.bfloat16, tag="win")
                s_t = io_pool.tile([128, 48, 32], mybir.dt.bfloat16, tag="s")
                win_src = win_ext[item].rearrange("p (g c) -> p g c", g=48)
                s_src = s_ext[item].rearrange("p (g t) -> p g t", g=48)
                nc.sync.dma_start(win_t[:], win_src[:])
                nc.sync.dma_start(s_t[:], s_src[:])
                out_t = out_pool.tile([128, 48, 4, 32], mybir.dt.bfloat16, tag="out")
                GRP = 16  # quad-groups per psum generation (16 * 32 = 512 cols)
                for gg in range(0, 48, GRP):
                    pss = [
                        psum_pool.tile(
                            [128, GRP * 32],
                            mybir.dt.float32,
                            tag=f"p{q}",
                            name=f"ps{q}",
                        )
                        for q in range(4)
                    ]
                    for q in range(4):
                        for i in range(GRP):
                            gq = gg + i
                            nc.tensor.matmul(
                                pss[q][:, i * 32 : (i + 1) * 32],
                                lhsT=win_t[32 * q : 32 * (q + 1), gq, :],
                                rhs=s_t[32 * q : 32 * (q + 1), gq, :],
                                start=True,
                                stop=True,
                            )
                    for q in range(4):
                        nc.vector.tensor_copy(
                            out_t[:, gg : gg + GRP, q, :],
                            pss[q][:].rearrange("p (i t) -> p i t", i=GRP),
                        )
                nc.sync.dma_start(
                    out_ext[item], out_t[:].rearrange("p g h t -> p (g h t)")
                )

            def duo_fused64(item0):
                # pair64 tiling; win+S arrive as one tensor, two items per DMA.
                WE = 48 * F
                ws_t = io_pool.tile(
                    [128, 2, WE + 48 * 64], mybir.dt.bfloat16, tag="ws"
                )
                nc.sync.dma_start(
                    ws_t[:], ws_ext[item0 : item0 + 2].rearrange("i p e -> p i e")
                )
                out_t = out_pool.tile(
                    [128, 2, 48, 2, 64], mybir.dt.bfloat16, tag="out"
                )
                GRP = 16
                for it in range(2):
                    win_v = ws_t[:, it, :WE].rearrange("p (g c) -> p g c", g=48)
                    s_v = ws_t[:, it, WE:].rearrange("p (g t) -> p g t", g=48)
                    for gg in range(0, 48, GRP):
                        ps_a = psum_pool.tile(
                            [128, GRP * 64], mybir.dt.float32, tag="pa"
                        )
                        ps_b = psum_pool.tile(
                            [128, GRP * 64], mybir.dt.float32, tag="pb"
                        )
                        for q in range(GRP):
                            gp = gg + q
                            nc.tensor.matmul(
                                ps_a[:, q * 64 : (q + 1) * 64],
                                lhsT=win_v[0:64, gp, :],
                                rhs=s_v[0:64, gp, :],
                                start=True,
                                stop=True,
                            )
                            nc.tensor.matmul(
                                ps_b[:, q * 64 : (q + 1) * 64],
                                lhsT=win_v[64:128, gp, :],
                                rhs=s_v[64:128, gp, :],
                                start=True,
                                stop=True,
                            )
                        nc.vector.tensor_copy(
                            out_t[:, it, gg : gg + GRP, 0, :],
                            ps_a[:].rearrange("p (q t) -> p q t", q=GRP),
                        )
                        nc.vector.tensor_copy(
                            out_t[:, it, gg : gg + GRP, 1, :],
                            ps_b[:].rearrange("p (q t) -> p q t", q=GRP),
                        )
                nc.sync.dma_start(
                    out_ext[item0 : item0 + 2].rearrange("i p t -> p i t"),
                    out_t[:].rearrange("p i g h t -> p i (g h t)"),
                )


            def item_pair64(item):
                # K=64 row-tiled matmuls: tile T0 (SBUF rows 0-63, even
                # output 64-col halves) and T8 (rows 64-127, odd halves)
                # run concurrently but MUST target different PSUM banks.
                win_t = io_pool.tile([128, 48, F], mybir.dt.bfloat16, tag="win")
                s_t = io_pool.tile([128, 48, 64], mybir.dt.bfloat16, tag="s")
                win_src = win_ext[item].rearrange("p (g c) -> p g c", g=48)
                s_src = s_ext[item].rearrange("p (g t) -> p g t", g=48)
                nc.sync.dma_start(win_t[:], win_src[:])
                nc.sync.dma_start(s_t[:], s_src[:])
                out_t = out_pool.tile([128, 48, 2, 64], mybir.dt.bfloat16, tag="out")
                GRP = 8  # pairs per psum bank-pair (8 * 64 cols = 512 each)
                for gg in range(0, 48, GRP):
                    ps_a = psum_pool.tile([128, GRP * 64], mybir.dt.float32, tag="pa")
                    ps_b = psum_pool.tile([128, GRP * 64], mybir.dt.float32, tag="pb")
                    for q in range(GRP):
                        gp = gg + q
                        nc.tensor.matmul(
                            ps_a[:, q * 64 : (q + 1) * 64],
                            lhsT=win_t[0:64, gp, :],
                            rhs=s_t[0:64, gp, :],
                            start=True,
                            stop=True,
                        )
                        nc.tensor.matmul(
                            ps_b[:, q * 64 : (q + 1) * 64],
                            lhsT=win_t[64:128, gp, :],
                            rhs=s_t[64:128, gp, :],
                            start=True,
                            stop=True,
                        )
                    nc.vector.tensor_copy(
                        out_t[:, gg : gg + GRP, 0, :],
                        ps_a[:].rearrange("p (q t) -> p q t", q=GRP),
                    )
                    nc.vector.tensor_copy(
                        out_t[:, gg : gg + GRP, 1, :],
                        ps_b[:].rearrange("p (q t) -> p q t", q=GRP),
                    )
                nc.sync.dma_start(
                    out_ext[item], out_t[:].rearrange("p g h t -> p (g h t)")
                )

            def pair_plain128(item0, n_chunks):
                """Process items item0, item0+1 with batched DMAs."""
                NCk = n_chunks
                win_t = io_pool.tile(
                    [128, 2, 48, NCk, F], mybir.dt.bfloat16, tag="win"
                )
                s_t = io_pool.tile(
                    [128, 2, 48, NCk, 128], mybir.dt.bfloat16, tag="s"
                )
                win_src = win_ext[item0 : item0 + 2].rearrange(
                    "i p (g j c) -> p i g j c", g=48, j=NCk
                )
                s_src = s_ext[item0 : item0 + 2].rearrange(
                    "i p (g j t) -> p i g j t", g=48, j=NCk
                )
                nc.sync.dma_start(win_t[:], win_src)
                nc.sync.dma_start(s_t[:], s_src)
                out_t = out_pool.tile([128, 2, T_OUT], mybir.dt.bfloat16, tag="out")
                GRP = 4
                for it in range(2):
                    for gg in range(0, 48, GRP):
                        ps = psum_pool.tile(
                            [128, GRP * 128], mybir.dt.float32, tag="ps"
                        )
                        for q in range(GRP):
                            g = gg + q
                            for j in range(NCk):
                                nc.tensor.matmul(
                                    ps[:, q * 128 : (q + 1) * 128],
                                    lhsT=win_t[:, it, g, j, :],
                                    rhs=s_t[:, it, g, j, :],
                                    start=(j == 0),
                                    stop=(j == NCk - 1),
                                )
                        nc.vector.tensor_copy(
                            out_t[:, it, gg * 128 : (gg + GRP) * 128], ps[:]
                        )
                nc.sync.dma_start(
                    out_ext[item0 : item0 + 2].rearrange("i p t -> p i t"), out_t[:]
                )

            def body(_iv=None):
                if mode == "fused64":
                    for item0 in range(0, ITEMS_PER_CORE, 2):
                        duo_fused64(item0)
                elif mode == "pair64":
                    for item in range(ITEMS_PER_CORE):
                        item_pair64(item)
                else:
                    for item0 in range(0, ITEMS_PER_CORE, 2):
                        pair_plain128(item0, int(mode.split("_")[1]))

            if n_reps == 1:
                body()
            else:
                with tc.For_i(0, n_reps, 1) as iv:
                    body(iv)

    _split_multiwait_instructions(nc)
    return nc


def get_program(mode: str, n_reps: int = 1, internal_io: bool = False):
    key = (mode, n_reps, internal_io)
    if key not in _PROGRAM_CACHE:
        _PROGRAM_CACHE[key] = build_program(mode, n_reps, internal_io)
    return _PROGRAM_CACHE[key]


# ---------------------------------------------------------------------------
# Entry point
# ---------------------------------------------------------------------------


def _prep(mel, vu_mask):
    mode, win_flat, s_flat = build_device_inputs(np.asarray(mel), np.asarray(vu_mask))
    in_maps = []
    for c in range(N_CORES):
        sl = slice(c * ITEMS_PER_CORE, (c + 1) * ITEMS_PER_CORE)
        if mode == "fused64":
            in_maps.append({"ws": np.ascontiguousarray(win_flat[sl])})
        else:
            in_maps.append(
                {
                    "win": np.ascontiguousarray(win_flat[sl]),
                    "s": np.ascontiguousarray(s_flat[sl]),
                }
            )
    return in_maps, mode


def run_on_device(in_maps, mode, n_reps: int = 1, trace: bool = False):
    from concourse.bass_utils import run_bass_kernel_spmd

    nc = get_program(mode, n_reps)
    return run_bass_kernel_spmd(nc, in_maps, core_ids=list(range(N_CORES)), trace=trace)


def run_timing(mode, n_reps: int = 1):
    """Run the internal-IO variant (garbage data, tiny transfers)."""
    from concourse.bass_utils import run_bass_kernel_spmd

    nc = get_program(mode, n_reps, internal_io=True)
    dummy = np.zeros((1, 1), np.float32)
    in_maps = [{"tin": dummy} for _ in range(N_CORES)]
    return run_bass_kernel_spmd(nc, in_maps, core_ids=list(range(N_CORES)))


def kernel(mel, vu_mask, F_out, T_out):
    mel = np.asarray(mel)
    vu_mask = np.asarray(vu_mask)
    assert int(F_out) == F and int(T_out) == T_OUT
    assert mel.shape == (B, 1, F, T_IN) and vu_mask.shape == (B, T_IN)

    in_maps, mode = _prep(mel, vu_mask)
    res = run_on_device(in_maps, mode)
    out = np.concatenate(
        [np.asarray(res.results[c]["out"]) for c in range(N_CORES)], axis=0
    )
    return out.reshape(B, 1, F, T_OUT).astype(np.float32)
